# revision 7
# baseline (speedup 1.0000x reference)
"""Trainium2 Bass kernel for nn_PolicyNetwork3x3 (tic-tac-toe policy/value net).

The network is tiny (conv 1->16 k2 on a 3x3 board -> fc 32 -> policy head with
masked softmax over 9 cells + tanh value head), so per the sharding hint we
replicate the whole program on all 8 NeuronCores and take core 0's output.

All linear algebra is restructured host-side into ONE packed matrix ("w",
[64 x 146] f32, which includes the board as a column) so the on-chip program
is a single input DMA, a chain of six small PE matmuls interleaved with
fused-bias activations on the scalar engine, and a single output DMA:

  - conv+im2col is folded into a single [9, 64] matrix L9 applied to the
    flattened board x9 [9, 1]  (out partition m = c*4 + i*2 + j).
  - fc / pre-head biases ride the following ACT instruction as per-partition
    bias APs; the pre-head weight blocks get an extra all-zero column whose
    bias entry is 1.0, so ReLU emits the constant-1 row that folds a2_b/v2_b
    into the final matmuls.
  - the final policy/value matmuls flip operands (lhsT = activations) so their
    outputs land on one partition ([1, 9] / [1, 1]) and the masked softmax
    runs along the free dimension.
  - the legality mask (-1e30 for occupied cells) is applied by accumulating a
    second matmul into the logits PSUM: lhsT = x9^2 [9,1], rhs = -1e30 * I9.
    exp(logit - 1e30) underflows to exactly 0.0f, matching avail*exp() == 0.
  - softmax skips the max-shift (logits are O(1); the shift cancels in the
    ratio); Exp + its free-dim sum is one ACT instruction via accum_out, and
    1/s is Exp(-Ln(s)) so the whole epilogue stays on ACT.

Raw Bass (no TileContext): the program is a straight line, so manual
semaphores are simple, every instruction carries at most one sync wait (the
PE LDWEIGHTS / DMA / CTRL slots encode very few), and we skip the Tile
kernel-tail drain + all-engine barrier entirely.
"""

import numpy as np

F32 = np.float32

# Packed matrix layout: w [64 partitions, 146 cols]
_C_L9 = 0        # cols   0:64  rows 0:9   conv-as-matmul   lhsT [9, 64]
_C_FC = 64       # cols  64:96  rows 0:64  fc_w.T           lhsT [64, 32]
_C_A1 = 96       # cols  96:113 rows 0:32  [a1_w.T | 0]     lhsT [32, 17]
_C_V1 = 113      # cols 113:122 rows 0:32  [v1_w.T | 0]     lhsT [32, 9]
_C_FCB = 122     # col  122     rows 0:32  fc_b
_C_A1B = 123     # col  123     rows 0:17  [a1_b; 1.0]
_C_V1B = 124     # col  124     rows 0:9   [v1_b; 1.0]
_C_A2 = 125      # cols 125:134 rows 0:17  [a2_w.T; a2_b]   rhs [17, 9]
_C_V2 = 134      # col  134     rows 0:9   [v2_w.T; v2_b]   rhs [9, 1]
_C_NEGI = 135    # cols 135:144 rows 0:9   -1e30 * eye(9)   rhs [9, 9]
_C_X = 144       # col  144     rows 0:9   flattened board  rhs [9, 1]
_C_ZERO = 145    # col  145     all rows   0.0 (bias for activations)
_W_COLS = 146

_MASK_BIG = F32(1e30)


def _pack_weights(conv_w, fc_w, fc_b, a1_w, a1_b, a2_w, a2_b,
                  v1_w, v1_b, v2_w, v2_b, x) -> np.ndarray:
    W = np.zeros((64, _W_COLS), F32)
    # conv (no bias): out[c*4 + i*2 + j] = sum_{di,dj} conv_w[c,0,di,dj] * x[i+di, j+dj]
    L9 = np.zeros((9, 64), F32)
    for c in range(16):
        for i in range(2):
            for j in range(2):
                m = c * 4 + i * 2 + j
                for di in range(2):
                    for dj in range(2):
                        L9[(i + di) * 3 + (j + dj), m] += conv_w[c, 0, di, dj]
    W[0:9, _C_L9:_C_L9 + 64] = L9
    W[0:64, _C_FC:_C_FC + 32] = fc_w.T
    W[0:32, _C_A1:_C_A1 + 16] = a1_w.T          # col _C_A1+16 stays 0
    W[0:32, _C_V1:_C_V1 + 8] = v1_w.T           # col _C_V1+8 stays 0
    W[0:32, _C_FCB] = fc_b
    W[0:16, _C_A1B] = a1_b
    W[16, _C_A1B] = 1.0                         # ReLU(0*y2r + 1) = constant-1 row
    W[0:8, _C_V1B] = v1_b
    W[8, _C_V1B] = 1.0
    W[0:16, _C_A2:_C_A2 + 9] = a2_w.T
    W[16, _C_A2:_C_A2 + 9] = a2_b
    W[0:8, _C_V2] = v2_w.reshape(8)
    W[8, _C_V2] = float(v2_b.reshape(-1)[0])
    W[0:9, _C_NEGI:_C_NEGI + 9] = -_MASK_BIG * np.eye(9, dtype=F32)
    W[0:9, _C_X] = x.reshape(9)
    return W


_NC_CACHE = None


def _build_nc():
    """Build the Bass program (once); cached across kernel() calls."""
    global _NC_CACHE
    if _NC_CACHE is not None:
        return _NC_CACHE

    from contextlib import ExitStack

    import concourse.bass as bass
    import concourse.mybir as mybir

    DT = mybir.dt.float32
    ACT = mybir.ActivationFunctionType

    nc = bass.Bass("TRN2", target_bir_lowering=False, debug=False)
    w_d = nc.dram_tensor("w", [64, _W_COLS], DT, kind="ExternalInput")
    o_d = nc.dram_tensor("out", [1, 10], DT, kind="ExternalOutput")

    with ExitStack() as ctx:
        en = ctx.enter_context
        W = en(nc.sbuf_tensor("W", [64, _W_COLS], DT))
        x9sq = en(nc.sbuf_tensor("x9sq", [9, 1], DT))
        yr = en(nc.sbuf_tensor("yr", [64, 1], DT))
        y2r = en(nc.sbuf_tensor("y2r", [32, 1], DT))
        a1r = en(nc.sbuf_tensor("a1r", [17, 1], DT))
        vr = en(nc.sbuf_tensor("vr", [9, 1], DT))
        e = en(nc.sbuf_tensor("e", [1, 9], DT))
        s = en(nc.sbuf_tensor("s", [1, 1], DT))
        ls = en(nc.sbuf_tensor("ls", [1, 1], DT))
        r = en(nc.sbuf_tensor("r", [1, 1], DT))
        outp = en(nc.sbuf_tensor("outp", [1, 10], DT))
        # one PSUM tensor per bank: no same-bank PE-write / ACT-read overlap
        p1 = en(nc.psum_tensor("p1", [64, 1], DT))
        p2 = en(nc.psum_tensor("p2", [32, 1], DT))
        p3a = en(nc.psum_tensor("p3a", [17, 1], DT))
        p3b = en(nc.psum_tensor("p3b", [9, 1], DT))
        p4 = en(nc.psum_tensor("p4", [1, 9], DT))
        p5 = en(nc.psum_tensor("p5", [1, 1], DT))
        dma_sem = en(nc.semaphore("dma_sem"))
        pe_sem = en(nc.semaphore("pe_sem"))
        act_sem = en(nc.semaphore("act_sem"))
        block = en(nc.Block())

        xcol = W[0:9, _C_X:_C_X + 1]

        def z(p):  # zero bias AP with p partitions
            return W[0:p, _C_ZERO:_C_ZERO + 1]

        @block.sync
        def _(sync):
            sync.dma_start(W[:], w_d[:]).then_inc(dma_sem, 16)
            sync.wait_ge(act_sem, 10)
            sync.dma_start(o_d[:], outp[:]).then_inc(dma_sem, 16)
            sync.wait_ge(dma_sem, 32)

        @block.tensor
        def _(pe):
            pe.wait_ge(dma_sem, 16)
            pe.matmul(p1[:], W[0:9, _C_L9:_C_L9 + 64], xcol,
                      start=True, stop=True).then_inc(pe_sem, 1)        # 1
            pe.wait_ge(act_sem, 2)
            pe.matmul(p2[:], W[0:64, _C_FC:_C_FC + 32], yr[:],
                      start=True, stop=True).then_inc(pe_sem, 1)        # 2
            pe.wait_ge(act_sem, 3)
            pe.matmul(p3a[:], W[0:32, _C_A1:_C_A1 + 17], y2r[:],
                      start=True, stop=True).then_inc(pe_sem, 1)        # 3
            pe.matmul(p3b[:], W[0:32, _C_V1:_C_V1 + 9], y2r[:],
                      start=True, stop=True).then_inc(pe_sem, 1)        # 4
            pe.wait_ge(act_sem, 4)
            pe.matmul(p4[:], a1r[:], W[0:17, _C_A2:_C_A2 + 9],
                      start=True, stop=False)
            pe.matmul(p4[:], x9sq[:], W[0:9, _C_NEGI:_C_NEGI + 9],
                      start=False, stop=True).then_inc(pe_sem, 1)       # 5
            pe.wait_ge(act_sem, 5)
            pe.matmul(p5[:], vr[:], W[0:9, _C_V2:_C_V2 + 1],
                      start=True, stop=True).then_inc(pe_sem, 1)        # 6

        @block.scalar
        def _(act):
            act.wait_ge(dma_sem, 16)
            act.activation(x9sq[:], xcol, ACT.Square,
                           bias=z(9)).then_inc(act_sem, 1)              # 1
            act.wait_ge(pe_sem, 1)
            act.activation(yr[:], p1[:], ACT.Relu,
                           bias=z(64)).then_inc(act_sem, 1)             # 2
            act.wait_ge(pe_sem, 2)
            act.activation(y2r[:], p2[:], ACT.Relu,
                           bias=W[0:32, _C_FCB:_C_FCB + 1]).then_inc(act_sem, 1)   # 3
            act.wait_ge(pe_sem, 3)
            act.activation(a1r[:], p3a[:], ACT.Relu,
                           bias=W[0:17, _C_A1B:_C_A1B + 1]).then_inc(act_sem, 1)   # 4
            act.wait_ge(pe_sem, 4)
            act.activation(vr[:], p3b[:], ACT.Relu,
                           bias=W[0:9, _C_V1B:_C_V1B + 1]).then_inc(act_sem, 1)    # 5
            act.wait_ge(pe_sem, 5)
            act.activation(e[:], p4[:], ACT.Exp, bias=z(1),
                           accum_out=s[:]).then_inc(act_sem, 1)         # 6
            # the ACT pipeline does not interlock back-to-back RAW through
            # SBUF, so self-wait on act_sem before each dependent read
            act.wait_ge(act_sem, 6)
            act.activation(ls[:], s[:], ACT.Ln,
                           bias=z(1)).then_inc(act_sem, 1)              # 7
            act.wait_ge(act_sem, 7)
            act.activation(r[:], ls[:], ACT.Exp, bias=z(1),
                           scale=-1.0).then_inc(act_sem, 1)             # 8
            act.wait_ge(act_sem, 8)
            act.activation(outp[0:1, 0:9], e[:], ACT.Copy,
                           scale=r[:]).then_inc(act_sem, 1)             # 9
            act.wait_ge(pe_sem, 6)
            act.activation(outp[0:1, 9:10], p5[:], ACT.Tanh,
                           bias=z(1)).then_inc(act_sem, 1)              # 10

    _NC_CACHE = nc
    return nc


def _run(inputs: dict, **run_kwargs):
    """Run on all 8 cores (replicated); returns BassKernelResults."""
    from concourse import bass_utils

    W = _pack_weights(
        inputs["conv_w"], inputs["fc_w"], inputs["fc_b"],
        inputs["a1_w"], inputs["a1_b"], inputs["a2_w"], inputs["a2_b"],
        inputs["v1_w"], inputs["v1_b"], inputs["v2_w"], inputs["v2_b"],
        inputs["x"],
    )
    nc = _build_nc()
    core_ids = list(range(8))
    in_maps = [{"w": W} for _ in core_ids]
    return bass_utils.run_bass_kernel_spmd(nc, in_maps, core_ids, **run_kwargs)


def kernel(**inputs):
    res = _run(inputs)
    out = res.results[0]["out"].reshape(10)
    prob = out[0:9].reshape(3, 3).astype(F32)
    value = out[9:10].reshape(1, 1).astype(F32)
    return prob, value


# revision 17
# speedup vs baseline: 1.2924x; 1.2924x over previous
"""Trainium2 Bass kernel for nn_PolicyNetwork3x3 (tic-tac-toe policy/value net).

The network is tiny (conv 1->16 k2 on a 3x3 board -> fc 32 -> policy head with
masked softmax over 9 cells + tanh value head), so per the sharding hint we
replicate the whole program on all 8 NeuronCores and take core 0's output.

All linear algebra is restructured host-side into ONE packed matrix ("w",
[64 x 160] f32, which includes the board as a column) so the on-chip program
is a single input DMA, five small PE matmuls interleaved with fused-bias
activations on the scalar engine, a 2-op DVE softmax normalize, and one
output DMA:

  - conv+im2col is folded into a single [9, 64] matrix L9 applied to the
    flattened board x9 [9, 1]  (out partition m = c*4 + i*2 + j).
  - both pre-heads run as ONE matmul [32 x 41] (policy rows 0:17, value rows
    32:41 so the later lhsT slices stay 32-partition-aligned) followed by ONE
    fused-bias ReLU; the extra all-zero weight columns with bias 1.0 emit the
    constant-1 rows that fold a2_b / v2_b into the final matmuls.
  - the final policy/value matmuls flip operands (lhsT = activations) so their
    outputs land on one partition ([1, 9] / [1, 1]) and the masked softmax
    runs along the free dimension.
  - the legality mask (-1e30 for occupied cells) is applied by accumulating a
    second matmul into the logits PSUM: lhsT = x9^2 [9,1], rhs = -1e30 * I9.
    exp(logit - 1e30) underflows to exactly 0.0f, matching avail*exp() == 0.
  - softmax skips the max-shift (logits are O(1); the shift cancels in the
    ratio); Exp + its free-dim sum is one ACT instruction via accum_out; the
    normalize is DVE reciprocal + tensor_scalar_mul.

Raw Bass, no Block and no TileContext: everything sits in the main basic
block with hand-placed semaphores, so there is no tile drain / all-engine
barrier, each instruction carries at most one sync wait (standalone wait_ge
instructions otherwise), and latency tricks are possible:

  - the weight DMA is triggered by the VECTOR engine, which reaches its
    main-block instructions ~1.4 us earlier than the sync sequencer, hiding
    most of the ~2.2 us DMA latency under the fixed NEFF preamble.
  - two dummy activations (Square, Tanh) run on ACT before the DMA wait to
    pull both 1.28 us ACT_TABLE_LOADs off the critical path.
  - the value head (tanh) completes while the PE still works on the policy
    logits, so only the softmax tail trails into the output DMA.
"""

import numpy as np

F32 = np.float32

# Packed matrix layout: w [64 partitions, 160 cols]
_C_L9 = 0        # cols   0:64  rows 0:9   conv-as-matmul    lhsT [9, 64]
_C_FC = 64       # cols  64:96  rows 0:64  fc_w.T            lhsT [64, 32]
_C_H = 96        # cols  96:137 rows 0:32  merged pre-heads  lhsT [32, 41]
#                  [a1_w.T | 0 | 0*15 | v1_w.T | 0]
_C_FCB = 137     # col  137     rows 0:32  fc_b
_C_HB = 138      # col  138     rows 0:41  [a1_b; 1; 0*15; v1_b; 1]
_C_A2 = 139      # cols 139:148 rows 0:17  [a2_w.T; a2_b]    rhs [17, 9]
_C_V2 = 148      # col  148     rows 0:9   [v2_w.T; v2_b]    rhs [9, 1]
_C_NEGI = 149    # cols 149:158 rows 0:9   -1e30 * eye(9)    rhs [9, 9]
_C_X = 158       # col  158     rows 0:9   flattened board   rhs [9, 1]
_C_ZERO = 159    # col  159     all rows   0.0 (bias for activations)
_W_COLS = 160

_MASK_BIG = F32(1e30)


def _pack_weights(conv_w, fc_w, fc_b, a1_w, a1_b, a2_w, a2_b,
                  v1_w, v1_b, v2_w, v2_b, x) -> np.ndarray:
    W = np.zeros((64, _W_COLS), F32)
    # conv (no bias): out[c*4 + i*2 + j] = sum_{di,dj} conv_w[c,0,di,dj] * x[i+di, j+dj]
    L9 = np.zeros((9, 64), F32)
    for c in range(16):
        for i in range(2):
            for j in range(2):
                m = c * 4 + i * 2 + j
                for di in range(2):
                    for dj in range(2):
                        L9[(i + di) * 3 + (j + dj), m] += conv_w[c, 0, di, dj]
    W[0:9, _C_L9:_C_L9 + 64] = L9
    W[0:64, _C_FC:_C_FC + 32] = fc_w.T
    W[0:32, _C_H:_C_H + 16] = a1_w.T            # cols 16..31 stay 0
    W[0:32, _C_H + 32:_C_H + 40] = v1_w.T       # col 40 stays 0
    W[0:32, _C_FCB] = fc_b
    W[0:16, _C_HB] = a1_b
    W[16, _C_HB] = 1.0                          # ReLU(0 + 1) = constant-1 row
    W[32:40, _C_HB] = v1_b
    W[40, _C_HB] = 1.0
    W[0:16, _C_A2:_C_A2 + 9] = a2_w.T
    W[16, _C_A2:_C_A2 + 9] = a2_b
    # value head rhs lives at rows 32:41 to match avr[32:41] (matmul operands
    # must share the same base partition)
    W[32:40, _C_V2] = v2_w.reshape(8)
    W[40, _C_V2] = float(v2_b.reshape(-1)[0])
    W[0:9, _C_NEGI:_C_NEGI + 9] = -_MASK_BIG * np.eye(9, dtype=F32)
    W[0:9, _C_X] = x.reshape(9)
    return W


_NC_CACHE = None


def _build_nc():
    """Build the Bass program (once); cached across kernel() calls."""
    global _NC_CACHE
    if _NC_CACHE is not None:
        return _NC_CACHE

    from contextlib import ExitStack

    import concourse.bass as bass
    import concourse.mybir as mybir

    DT = mybir.dt.float32
    ACT = mybir.ActivationFunctionType

    nc = bass.Bass("TRN2", target_bir_lowering=False, debug=False)
    w_d = nc.dram_tensor("w", [64, _W_COLS], DT, kind="ExternalInput")
    o_d = nc.dram_tensor("out", [1, 10], DT, kind="ExternalOutput")

    with ExitStack() as ctx:
        en = ctx.enter_context
        W = en(nc.sbuf_tensor("W", [64, _W_COLS], DT))
        scr = en(nc.sbuf_tensor("scr", [1, 1], DT))    # ACT table-warm scratch
        scro = en(nc.sbuf_tensor("scro", [1, 2], DT))  # warmup outputs (unused)
        x9sq = en(nc.sbuf_tensor("x9sq", [9, 1], DT))
        yr = en(nc.sbuf_tensor("yr", [64, 1], DT))
        y2r = en(nc.sbuf_tensor("y2r", [32, 1], DT))
        avr = en(nc.sbuf_tensor("avr", [41, 1], DT))   # [a1r;1;pad | vr;1]
        e = en(nc.sbuf_tensor("e", [1, 9], DT))
        s = en(nc.sbuf_tensor("s", [1, 1], DT))
        r = en(nc.sbuf_tensor("r", [1, 1], DT))
        outp = en(nc.sbuf_tensor("outp", [1, 10], DT))
        # one PSUM tensor per bank: no same-bank PE-write / ACT-read overlap
        p1 = en(nc.psum_tensor("p1", [64, 1], DT))
        p2 = en(nc.psum_tensor("p2", [32, 1], DT))
        p3 = en(nc.psum_tensor("p3", [41, 1], DT))
        p4 = en(nc.psum_tensor("p4", [1, 9], DT))
        p5 = en(nc.psum_tensor("p5", [1, 1], DT))
        dma_sem = en(nc.semaphore("dma_sem"))
        pe_sem = en(nc.semaphore("pe_sem"))
        act_sem = en(nc.semaphore("act_sem"))
        dve_sem = en(nc.semaphore("dve_sem"))
        ws_sem = en(nc.semaphore("ws_sem"))

        xcol = W[0:9, _C_X:_C_X + 1]

        def z(p):  # zero bias AP with p partitions
            return W[0:p, _C_ZERO:_C_ZERO + 1]

        # ---- DVE: normalize the softmax at the end ----
        nc.vector.wait_ge(act_sem, 6)
        nc.vector.reciprocal(r[:], s[:]).then_inc(dve_sem, 1)
        nc.vector.wait_ge(dve_sem, 1)   # DVE pipeline: RAW on r needs a wait
        nc.vector.tensor_scalar_mul(outp[0:1, 0:9], e[:],
                                    r[:]).then_inc(dve_sem, 1)

        # ---- ACT: trigger the weight DMA (ACT reaches its main-block
        #      instructions earlier than SP), warm both PWP tables during the
        #      DMA flight, then run the fused-bias activation chain ----
        nc.gpsimd.memset(scr[:], 0.0).then_inc(ws_sem, 1)
        nc.scalar.dma_start(W[:], w_d[:]).then_inc(dma_sem, 16)
        nc.scalar.wait_ge(ws_sem, 1)
        nc.scalar.activation(scro[0:1, 0:1], scr[:], ACT.Square, bias=scr[:])
        nc.scalar.activation(scro[0:1, 1:2], scr[:], ACT.Tanh, bias=scr[:])
        nc.scalar.wait_ge(dma_sem, 16)
        nc.scalar.activation(x9sq[:], xcol, ACT.Square,
                             bias=z(9)).then_inc(act_sem, 1)            # 1
        nc.scalar.wait_ge(pe_sem, 1)
        nc.scalar.activation(yr[:], p1[:], ACT.Relu,
                             bias=z(64)).then_inc(act_sem, 1)           # 2
        nc.scalar.wait_ge(pe_sem, 2)
        nc.scalar.activation(y2r[:], p2[:], ACT.Relu,
                             bias=W[0:32, _C_FCB:_C_FCB + 1]).then_inc(act_sem, 1)  # 3
        nc.scalar.wait_ge(pe_sem, 3)
        nc.scalar.activation(avr[:], p3[:], ACT.Relu,
                             bias=W[0:41, _C_HB:_C_HB + 1]).then_inc(act_sem, 1)    # 4
        nc.scalar.wait_ge(pe_sem, 4)
        nc.scalar.activation(outp[0:1, 9:10], p5[:], ACT.Tanh,
                             bias=z(1)).then_inc(act_sem, 1)            # 5
        nc.scalar.wait_ge(pe_sem, 5)
        nc.scalar.activation(e[:], p4[:], ACT.Exp, bias=z(1),
                             accum_out=s[:]).then_inc(act_sem, 1)       # 6

        # ---- PE: five matmuls ----
        nc.tensor.wait_ge(dma_sem, 16)
        nc.tensor.matmul(p1[:], W[0:9, _C_L9:_C_L9 + 64], xcol,
                         start=True, stop=True).then_inc(pe_sem, 1)     # 1
        nc.tensor.wait_ge(act_sem, 2)
        nc.tensor.matmul(p2[:], W[0:64, _C_FC:_C_FC + 32], yr[:],
                         start=True, stop=True).then_inc(pe_sem, 1)     # 2
        nc.tensor.wait_ge(act_sem, 3)
        nc.tensor.matmul(p3[:], W[0:32, _C_H:_C_H + 41], y2r[:],
                         start=True, stop=True).then_inc(pe_sem, 1)     # 3
        nc.tensor.wait_ge(act_sem, 4)
        nc.tensor.matmul(p5[:], avr[32:41, :], W[32:41, _C_V2:_C_V2 + 1],
                         start=True, stop=True).then_inc(pe_sem, 1)     # 4
        nc.tensor.matmul(p4[:], avr[0:17, :], W[0:17, _C_A2:_C_A2 + 9],
                         start=True, stop=False)
        nc.tensor.matmul(p4[:], x9sq[:], W[0:9, _C_NEGI:_C_NEGI + 9],
                         start=False, stop=True).then_inc(pe_sem, 1)    # 5

        # ---- SP: output DMA once value (act 5) and prob (dve 2) landed ----
        nc.sync.wait_ge(act_sem, 5)
        nc.sync.wait_ge(dve_sem, 2)
        nc.sync.dma_start(o_d[:], outp[:]).then_inc(dma_sem, 16)
        nc.sync.wait_ge(dma_sem, 32)

    _NC_CACHE = nc
    return nc


def _run(inputs: dict, **run_kwargs):
    """Run on all 8 cores (replicated); returns BassKernelResults."""
    from concourse import bass_utils

    W = _pack_weights(
        inputs["conv_w"], inputs["fc_w"], inputs["fc_b"],
        inputs["a1_w"], inputs["a1_b"], inputs["a2_w"], inputs["a2_b"],
        inputs["v1_w"], inputs["v1_b"], inputs["v2_w"], inputs["v2_b"],
        inputs["x"],
    )
    nc = _build_nc()
    core_ids = list(range(8))
    in_maps = [{"w": W} for _ in core_ids]
    return bass_utils.run_bass_kernel_spmd(nc, in_maps, core_ids, **run_kwargs)


def kernel(**inputs):
    res = _run(inputs)
    out = res.results[0]["out"].reshape(10)
    prob = out[0:9].reshape(3, 3).astype(F32)
    value = out[9:10].reshape(1, 1).astype(F32)
    return prob, value
